# revision 1
# baseline (speedup 1.0000x reference)
"""Trainium2 Bass kernel for nn_ConvAttn (BN -> 3x(conv1d+linear+relu) -> scalar MHA -> linear).

Sharding: pure data parallel, batch 64 -> 8 cores x 8 batches.

Math folds (host side):
  - BatchNorm (eval) -> per-position affine xn = x*s + t
  - conv bias + linear bias -> extra all-ones "j=998" row trick: lwT row 998 = lin_b,
    y[:, 998] forced to 1.0 (lwT row 999 = 0 kills the garbage j=999 lane)
  - in_proj q,k affines -> fused into exp: exp(q_i*k_j) = Exp(scale_j * r_i + bias_j)
    with r = relu(Q+lb), scale_j = w0*k_j, bias_j = b0*k_j, k_j = w1*relu(K^T)_j + b1
  - in_proj v affine + out_proj -> folded into owt/outb:
      ctx' = (ow*w2)*num/den + (ow*b2+obp);  out = ctx' @ out_w.T + out_b
      => owt[i,d] = out_w[d,i]*ow*w2, outb[d] = out_b[d] + (ow*b2+obp)*sum_i out_w[d,i]

Device pipeline per core (8 batches):
  phase 0: DMA x/s/t tiled [64, 506] ((b,jc) partition layout), BN + 10-tap conv on DVE
  phase 1: PE transpose y -> yT [125, 64]; Q = y@lwT_q ([8,1000] psum);
           K^T, V^T per d-chunk [125, 8] (weights stationary)
  phase 2: per batch: PE ones-outer broadcast r_b -> psum [125,1000];
           8x ACT Exp (fused k-scale/bias) -> S^T chunks [125,1000] sbuf;
           64x PE reduce (S^T stationary, [v,1] moving) -> num/den [125,2];
           DVE reciprocal+mult -> ctxT [125, 64]
  phase 3: PE out matmul (ctxT stationary, owt moving) + DVE bias add -> out [8, 1000]
"""

import numpy as np

B = 64
L_IN = 4001
LC = 998
D = 1000
EPS = 1e-5
NCORE = 8
BPC = B // NCORE            # 8 batches per core
PADL = 4016                 # padded x row length
W = 512                     # conv input window per jc chunk (needs 4*124+9+1; padded so tap slices of 500 fit)
NJC = 8                     # position chunks, stride 500
JCH = 125                   # conv outputs per chunk; 8*125 = 1000 lanes (998 valid + bias row + zero row)
NDC = 8                     # d chunks of 125 for K/V/attention
DCH = 125

_CACHE = {}


def _build_module():
    import concourse.bass as bass
    import concourse.bacc as bacc
    import concourse.mybir as mybir
    import concourse.tile as tile
    from concourse.masks import make_identity
    from contextlib import ExitStack

    fp32 = mybir.dt.float32
    bf16 = mybir.dt.bfloat16
    AF = mybir.ActivationFunctionType
    OP = mybir.AluOpType

    nc = bacc.Bacc()
    xst_d = nc.declare_dram_parameter("xst", [64, 3 * W], fp32, isOutput=False)
    cw_d = nc.declare_dram_parameter("cw", [64, 40], fp32, isOutput=False)
    pv_d = nc.declare_dram_parameter("pv", [125, 4], fp32, isOutput=False)
    lw_d = [
        nc.declare_dram_parameter(f"lw{n}", [1000, D], bf16, isOutput=False)
        for n in "qkv"
    ]
    lb_d = nc.declare_dram_parameter("lb", [1, 3072], fp32, isOutput=False)
    owt_d = nc.declare_dram_parameter("owt", [D, D], bf16, isOutput=False)
    ob_d = nc.declare_dram_parameter("ob", [BPC, D], fp32, isOutput=False)
    out_d = nc.declare_dram_parameter("out", [BPC, D], fp32, isOutput=True)

    with tile.TileContext(nc) as tc, ExitStack() as ctx:
        const = ctx.enter_context(tc.tile_pool(name="const", bufs=1))
        work = ctx.enter_context(tc.tile_pool(name="work", bufs=3))
        lwp = ctx.enter_context(tc.tile_pool(name="lwp", bufs=2))
        spool = ctx.enter_context(tc.tile_pool(name="spool", bufs=10))
        # PSUM budget (8 banks): q/out 2 + small(tr/kv/nd) 3x1 = 5
        ps_big = ctx.enter_context(tc.tile_pool(name="ps_big", bufs=1, space="PSUM"))
        ps_sm = ctx.enter_context(tc.tile_pool(name="ps_sm", bufs=3, space="PSUM"))
        ps_qb = ctx.enter_context(tc.tile_pool(name="ps_qb", bufs=1, space="PSUM"))

        # ---- constant-ish tiles ----
        xst = const.tile([64, 3 * W], fp32)
        cw_sb = const.tile([64, 40], fp32)
        pv_sb = const.tile([125, 4], fp32)
        ident = const.tile([64, 64], fp32)
        ones_row = const.tile([1, 128], fp32)
        r_sb = const.tile([BPC, D], fp32)
        sc_all = const.tile([125, 64], fp32)   # col = 8*dc + b : w0*k_j
        bi_all = const.tile([125, 64], fp32)   # col = 8*dc + b : b0*k_j
        vones = const.tile([125, 16 * NDC], fp32)  # per dc: 16 cols, interleaved (v_b, 1)
        ctxT = const.tile([125, 64], bf16)     # col = 8*ic + b
        outb_sb = const.tile([BPC, D], fp32)
        out_sb = const.tile([BPC, D], fp32)
        yT = [const.tile([JCH, 64], bf16, name=f"yT{n}") for n in "qkv"]

        nc.sync.dma_start(out=xst[:, :], in_=xst_d[:, :])
        nc.sync.dma_start(out=cw_sb[:, :], in_=cw_d[:, :])
        nc.sync.dma_start(out=pv_sb[:, :], in_=pv_d[:, :])
        lb_sb = const.tile([1, 3072], fp32)
        nc.sync.dma_start(out=lb_sb[:, :], in_=lb_d[:, :])
        nc.sync.dma_start(out=outb_sb[:, :], in_=ob_d[:, :])
        make_identity(nc, ident[:, :])
        nc.vector.memset(ones_row[:, :], 1.0)

        # ---- phase 0: BN + conv ----
        xn = const.tile([64, W], fp32)
        nc.vector.tensor_tensor(xn[:, :], xst[:, 0:W], xst[:, W : 2 * W], OP.mult)
        nc.vector.tensor_tensor(xn[:, :], xn[:, :], xst[:, 2 * W : 3 * W], OP.add)

        def xn_tap(tp):
            # [64, 125] view of xn at offset tp with free step 4
            v = xn[:, tp : tp + 500]
            return v.rearrange("p (j f) -> p j f", f=4)[:, :, 0:1].squeeze(2)

        y_tiles = []
        for ci in range(3):
            y = work.tile([64, JCH], fp32, name=f"y{ci}", tag=f"y{ci}")
            nc.vector.tensor_scalar(
                y[:, :], xn_tap(0),
                cw_sb[:, 10 * ci : 10 * ci + 1], cw_sb[:, 30 + ci : 31 + ci],
                OP.mult, OP.add,
            )
            for tp in range(1, 10):
                tmp = work.tile([64, JCH], fp32, name=f"tmp{ci}_{tp}", tag="convtmp")
                nc.vector.tensor_scalar(
                    tmp[:, :], xn_tap(tp),
                    cw_sb[:, 10 * ci + tp : 10 * ci + tp + 1], None, OP.mult,
                )
                nc.vector.tensor_tensor(y[:, :], y[:, :], tmp[:, :], OP.add)
            y_tiles.append(y)

        # ---- phase 1: transposes + QKV matmuls ----
        for ci in range(3):
            tr = ps_sm.tile([JCH, 64], fp32, name=f"tr{ci}", tag="small")
            nc.tensor.transpose(tr[:, :], y_tiles[ci][:, :], ident[:, :])
            nc.vector.tensor_copy(yT[ci][:, :], tr[:, :])

        def yT_cols(ci, jc):
            # [125, 8] view: columns jc, jc+8, ..., jc+56  (= y^T[jr, b] for chunk jc)
            return (
                yT[ci][:, :]
                .rearrange("p (b jc) -> p b jc", jc=8)[:, :, jc : jc + 1]
                .squeeze(2)
            )

        # Q/K/V linear: [8, 1000] psum each; y^T stationary, lwT moving (chunks
        # streamed, each consumed once).
        relu_sb = {0: r_sb}
        for ci, name in enumerate("qkv"):
            acc = ps_big.tile([BPC, 1024], fp32, name=f"acc{name}", tag="big")
            for jc in range(NJC):
                lwt = lwp.tile([JCH, D], bf16, name=f"lw{name}{jc}", tag="lw")
                nc.sync.dma_start(
                    out=lwt[:, :], in_=lw_d[ci][JCH * jc : JCH * (jc + 1), :]
                )
                for n0, n1 in ((0, 512), (512, 1000)):
                    nc.tensor.matmul(
                        acc[:, n0:n1], yT_cols(ci, jc), lwt[:, n0:n1],
                        start=(jc == 0), stop=False,
                    )
            for n0, n1 in ((0, 512), (512, 1000)):
                nc.tensor.matmul(
                    acc[:, n0:n1], ones_row[:, 0:BPC],
                    lb_sb[:, 1024 * ci + n0 : 1024 * ci + n1],
                    start=False, stop=True,
                )
            if ci == 0:
                nc.scalar.activation(r_sb[:, :], acc[0:BPC, 0:D], AF.Relu)
            else:
                kv_sb = const.tile([BPC, D], fp32, name=f"relu{name}")
                nc.scalar.activation(kv_sb[:, :], acc[0:BPC, 0:D], AF.Relu)
                relu_sb[ci] = kv_sb

        # transpose relu(K), relu(V) chunk-wise to [125, 8]; fold in_proj affines
        for ci in (1, 2):
            for dc in range(NDC):
                kt = ps_sm.tile([DCH, BPC], fp32, name=f"kt{ci}_{dc}", tag="small")
                nc.tensor.transpose(
                    kt[:, :],
                    relu_sb[ci][:, DCH * dc : DCH * (dc + 1)],
                    ident[0:BPC, 0:BPC],
                )
                if ci == 1:
                    nc.vector.tensor_scalar(
                        sc_all[:, 8 * dc : 8 * dc + 8], kt[:, :],
                        pv_sb[:, 0:1], pv_sb[:, 1:2], OP.mult, OP.add,
                    )
                    nc.vector.tensor_scalar(
                        bi_all[:, 8 * dc : 8 * dc + 8], kt[:, :],
                        pv_sb[:, 2:3], pv_sb[:, 3:4], OP.mult, OP.add,
                    )
                else:
                    vv = vones[:, 16 * dc : 16 * dc + 16]
                    vview = vv.rearrange("p (b two) -> p b two", two=2)
                    nc.vector.tensor_copy(vview[:, :, 0:1].squeeze(2), kt[:, :])
                    nc.vector.memset(vview[:, :, 1:2], 1.0)

        # ---- phase 2: attention ----
        for b in range(BPC):
            rrow = work.tile([1, D], fp32, name=f"rrow{b}", tag="rrow")
            nc.sync.dma_start(out=rrow[:, :], in_=r_sb[b : b + 1, :])
            qb = ps_qb.tile([DCH, 1024], fp32, name=f"qb{b}", tag="qb")
            for n0, n1 in ((0, 512), (512, 1000)):
                nc.tensor.matmul(
                    qb[:, n0:n1], ones_row[:, 0:DCH], rrow[:, n0:n1],
                    start=True, stop=True,
                )
            s_tiles = []
            for dc in range(NDC):
                st_ = spool.tile([DCH, D], fp32, name=f"s{b}_{dc}", tag="sS")
                col = 8 * dc + b
                nc.scalar.activation(
                    st_[:, :], qb[0:DCH, 0:D], AF.Exp,
                    bias=bi_all[:, col : col + 1], scale=sc_all[:, col : col + 1],
                )
                s_tiles.append(st_)
            for ic in range(8):
                nd = ps_sm.tile([DCH, 2], fp32, name=f"nd{b}_{ic}", tag="small")
                for dc in range(NDC):
                    rhs = vones[:, 16 * dc + 2 * b : 16 * dc + 2 * b + 2]
                    nc.tensor.matmul(
                        nd[:, :], s_tiles[dc][:, DCH * ic : DCH * (ic + 1)], rhs,
                        start=(dc == 0), stop=(dc == NDC - 1),
                    )
                rec = work.tile([DCH, 1], fp32, name=f"rec{b}_{ic}", tag="rec")
                nc.vector.reciprocal(rec[:, :], nd[:, 1:2])
                col = 8 * ic + b
                nc.vector.tensor_tensor(
                    ctxT[:, col : col + 1], nd[:, 0:1], rec[:, :], OP.mult
                )

        # ---- phase 3: output matmul + bias ----
        ow_sb = []
        for ic in range(8):
            t_ = lwp.tile([DCH, D], bf16, name=f"ow{ic}", tag="owt")
            nc.sync.dma_start(out=t_[:, :], in_=owt_d[DCH * ic : DCH * (ic + 1), :])
            ow_sb.append(t_)
        o_ps = ps_big.tile([BPC, 1024], fp32, name="o_ps", tag="big")
        for ic in range(8):
            for n0, n1 in ((0, 512), (512, 1000)):
                nc.tensor.matmul(
                    o_ps[:, n0:n1], ctxT[:, 8 * ic : 8 * ic + 8], ow_sb[ic][:, n0:n1],
                    start=(ic == 0), stop=(ic == 7),
                )
        nc.vector.tensor_tensor(out_sb[:, :], o_ps[0:BPC, 0:D], outb_sb[:, :], OP.add)
        nc.sync.dma_start(out=out_d[:, :], in_=out_sb[:, :])

    nc.compile()
    return nc


def _prep_inputs(inputs):
    import ml_dtypes
    f32 = np.float32
    bf = ml_dtypes.bfloat16
    x = np.ascontiguousarray(inputs["x"].reshape(B, L_IN)).astype(f32, copy=False)
    s = (inputs["bn_gamma"] / np.sqrt(inputs["bn_var"] + EPS)).astype(f32)
    t = (inputs["bn_beta"] - inputs["bn_mean"] * s).astype(f32)

    idx = (500 * np.arange(NJC))[:, None] + np.arange(W)[None, :]  # [8, 506]
    x_pad = np.zeros((B, PADL), f32)
    x_pad[:, :L_IN] = x
    xw = x_pad[:, idx]  # [64, 8, 506]

    s_pad = np.zeros(PADL, f32)
    s_pad[:L_IN] = s
    t_pad = np.zeros(PADL, f32)
    t_pad[:L_IN] = t
    s_t = np.tile(s_pad[idx], (BPC, 1))
    t_t = np.tile(t_pad[idx], (BPC, 1))

    cw = np.zeros(40, f32)
    for ci, n in enumerate("qkv"):
        cw[10 * ci : 10 * ci + 10] = inputs[f"conv_w_{n}"].reshape(10)
        cw[30 + ci] = inputs[f"conv_b_{n}"].reshape(())
    cw = np.tile(cw, (64, 1))

    w = inputs["in_proj_w"].reshape(3).astype(np.float64)
    bb = inputs["in_proj_b"].reshape(3).astype(np.float64)
    pv = np.tile(
        np.array([w[0] * w[1], w[0] * bb[1], bb[0] * w[1], bb[0] * bb[1]], f32),
        (125, 1),
    )

    lw = {}
    for ci, n in enumerate("qkv"):
        m = np.zeros((1000, D), f32)
        m[:LC, :] = inputs[f"lin_w_{n}"].T
        lw[n] = np.ascontiguousarray(m.astype(bf))

    lb_pack = np.zeros((1, 3072), f32)
    for ci, n in enumerate("qkv"):
        lb_pack[0, 1024 * ci : 1024 * ci + D] = inputs[f"lin_b_{n}"]

    ow = float(inputs["out_proj_w"].reshape(()))
    obp = float(inputs["out_proj_b"].reshape(()))
    out_w = inputs["out_w"].astype(f32)
    owt = np.ascontiguousarray((out_w.T * f32(ow * w[2])).astype(bf))
    outb = inputs["out_b"] + f32(ow * bb[2] + obp) * out_w.sum(axis=1)
    ob8 = np.tile(outb.astype(f32), (BPC, 1))

    shared = {
        "cw": np.ascontiguousarray(cw),
        "pv": np.ascontiguousarray(pv),
        "lwq": lw["q"],
        "lwk": lw["k"],
        "lwv": lw["v"],
        "lb": lb_pack,
        "owt": owt,
        "ob": np.ascontiguousarray(ob8),
    }
    in_maps = []
    for c in range(NCORE):
        m = dict(shared)
        xc = xw[BPC * c : BPC * (c + 1)].reshape(64, W)
        m["xst"] = np.ascontiguousarray(np.concatenate([xc, s_t, t_t], axis=1))
        in_maps.append(m)
    return in_maps


def kernel(**inputs) -> np.ndarray:
    from concourse.bass_utils import run_bass_kernel_spmd

    if "nc" not in _CACHE:
        _CACHE["nc"] = _build_module()
    nc = _CACHE["nc"]
    in_maps = _prep_inputs(inputs)
    res = run_bass_kernel_spmd(nc, in_maps, list(range(NCORE)))
    outs = [res.results[c]["out"] for c in range(NCORE)]
    return np.concatenate(outs, axis=0).reshape(B, 1, D).astype(np.float32)



# revision 15
# speedup vs baseline: 1.3578x; 1.3578x over previous
"""Trainium2 Bass kernel for nn_ConvAttn (BN -> 3x(conv1d+linear+relu) -> scalar MHA -> linear).

Sharding: pure data parallel, batch 64 -> 8 cores x 8 batches.

Key insight: with embed_dim=1 the attention context for query i depends on q_i
only through the scalar q_i:
    ctx_i = f(q_i),  f(t) = sum_j v_j e^{t k'_j} / sum_j e^{t k'_j}
(k' = k - mean(k); the mean shift cancels in softmax). f is smooth, so instead
of the 1000x1000 score matrix per batch we evaluate f at 64 nodes t_m spanning
the realized q-range and piecewise-linearly interpolate. PL interpolation in
the relu basis: f_PL(r) = f_0 + s_0 r + sum_m g_m relu(r - rho_m), so the
per-i evaluation is ONE matmul against a relu-basis matrix built by a single
ACT Relu. Offline validation: final rel err ~6e-5 vs exact.

Device pipeline per core (8 batches):
  P0: BN + 10-tap conv on DVE -> y [64,125] x3 (+ bias-lane trick: y-lane 998=1,
      999=0; lw row 998 = lin_b so linear bias rides the same matmuls)
  P1: PE transpose y -> yT; QKV linear (weights moving, bf16) -> [8,1000] psum;
      ACT Relu -> Qr/Kr/Vr rows [8,1000]
  P1.5: row stats (rmax, ksum) -> per-batch grid scalars; k'-fold; ln|v|+C row
  P2 per batch: 2 fp32r outer matmuls -> node exp inputs [128,1000]
      (rows 0:64 = t_m k' + lnv for numerator, 64:128 = t_m k' for denom);
      ACT Exp with accum_out -> node sums [128,1]; PE transpose -> [1,128];
      ~10 tiny DVE ops -> PL coeff vector g [64,1];
      1 fp32r matmul + ACT Relu -> relu-basis stack [64,1000];
      8 tiny matmuls (4cyc) -> ctx chunk cols [125,1] in psum
  P3: ctx -> bf16 ctxT [125,64]; out matmul (owt moving bf16) + bias; DMA out
"""

import numpy as np

B = 64
L_IN = 4001
LC = 998
D = 1000
EPS = 1e-5
NCORE = 8
BPC = B // NCORE            # 8 batches per core
PADL = 4016                 # padded x row length
W = 512                     # conv input window per jc chunk
NJC = 8                     # position chunks, stride 500
JCH = 125                   # conv outputs per chunk; 8*125 = 1000 lanes
NN = 64                     # interpolation nodes per batch (grid [0, rmax])
NK = NN - 2                 # interior relu knots m=1..62

_CACHE = {}


def _build_module(sc):
    import concourse.bass as bass
    import concourse.bacc as bacc
    import concourse.mybir as mybir
    import concourse.tile as tile
    from concourse.masks import make_identity
    from contextlib import ExitStack

    fp32 = mybir.dt.float32
    f32r = mybir.dt.float32r
    bf16 = mybir.dt.bfloat16
    AF = mybir.ActivationFunctionType
    OP = mybir.AluOpType
    AX = mybir.AxisListType

    # host-folded scalars (baked into the module as immediates)
    w0, b0 = sc["w0"], sc["b0"]
    w1 = sc["w1"]
    alpha, beta = sc["alpha"], sc["beta"]     # ln arg = alpha*Vr + beta
    fs1, fs2 = sc["fs1"], sc["fs2"]           # f' = fs1*(n/d) + fs2

    nc = bacc.Bacc()
    xst_d = nc.declare_dram_parameter("xst", [64, 3 * W], fp32, isOutput=False)
    cw_d = nc.declare_dram_parameter("cw", [64, 48], fp32, isOutput=False)
    lw_d = [
        nc.declare_dram_parameter(f"lw{n}", [1000, D], bf16, isOutput=False)
        for n in "qkv"
    ]
    owt_d = nc.declare_dram_parameter("owt", [D, D], bf16, isOutput=False)
    ob_d = nc.declare_dram_parameter("ob", [BPC, D], fp32, isOutput=False)
    # packed consts: [0:64]=0..63 | [64:128]=[1..62,0,0] | [128:256]=nmask | [256:320]=e63
    cr_d = nc.declare_dram_parameter("crows", [1, 320], f32r, isOutput=False)
    yfix_d = nc.declare_dram_parameter("yfix", [2, 8], bf16, isOutput=False)
    out_d = nc.declare_dram_parameter("out", [BPC, D], fp32, isOutput=True)

    with tile.TileContext(nc) as tc, ExitStack() as ctx:
        const = ctx.enter_context(tc.tile_pool(name="const", bufs=1))
        work = ctx.enter_context(tc.tile_pool(name="work", bufs=3))
        lwp = ctx.enter_context(tc.tile_pool(name="lwp", bufs=2))
        rowp = ctx.enter_context(tc.tile_pool(name="rowp", bufs=2))
        nexp = ctx.enter_context(tc.tile_pool(name="nexp", bufs=2))
        # PSUM (8 banks): data 2x2 + big(qkv/out) 2 + ctx 1 + small 1
        ps_data = ctx.enter_context(tc.tile_pool(name="ps_data", bufs=2, space="PSUM"))
        ps_big = ctx.enter_context(tc.tile_pool(name="ps_big", bufs=1, space="PSUM"))
        ps_sm = ctx.enter_context(tc.tile_pool(name="ps_sm", bufs=1, space="PSUM"))
        ps_ctx = ctx.enter_context(tc.tile_pool(name="ps_ctx", bufs=1, space="PSUM"))

        # ---- constants ----
        xst = const.tile([64, 3 * W], fp32)
        cw_sb = const.tile([64, 48], fp32)
        crows = const.tile([1, 320], f32r)
        ident = const.tile([128, 128], fp32)
        nc.sync.dma_start(out=xst[:, :], in_=xst_d[:, :])
        nc.sync.dma_start(out=cw_sb[:, :], in_=cw_d[:, :])
        nc.sync.dma_start(out=crows[:, :], in_=cr_d[:, :])
        make_identity(nc, ident[:, :])
        mfull = crows[:, 0:64].bitcast(fp32)    # 0..63
        mshift = crows[:, 64:128].bitcast(fp32)  # 1..62, 0, 0
        nmask = crows[:, 128:256]                # 1 x64, 0 x64 (f32r)
        ones64 = crows[:, 128:192]               # 1 x64 (f32r)
        ones8 = crows[:, 128:136].bitcast(fp32)  # 1 x8
        e63 = crows[:, 256:320].bitcast(fp32)    # 0 x63, 1
        scl64 = cw_sb[:, 40:41]         # col: 1 x63, 0  (relu-scale; row63 -> ones)
        outb_sb = const.tile([BPC, D], fp32)
        nc.sync.dma_start(out=outb_sb[:, :], in_=ob_d[:, :])

        ctxT = const.tile([JCH, 64], bf16)
        qr = const.tile([BPC, D], fp32, name="qr")
        kp = const.tile([BPC, D], fp32, name="kp")
        lnv = const.tile([BPC, D], fp32, name="lnv")
        scrow = const.tile([2, BPC], fp32, name="scrow")
        w0hb = const.tile([1, BPC], fp32, name="w0hb")
        neghb = const.tile([1, BPC], fp32, name="neghb")
        invh = const.tile([1, BPC], fp32, name="invh")

        # ---- P0: BN + conv ----
        xn = const.tile([64, W], fp32)
        nc.vector.tensor_tensor(xn[:, :], xst[:, 0:W], xst[:, W : 2 * W], OP.mult)
        nc.vector.tensor_tensor(xn[:, :], xn[:, :], xst[:, 2 * W : 3 * W], OP.add)

        def xn_tap(tp):
            v = xn[:, tp : tp + 500]
            return v.rearrange("p (j f) -> p j f", f=4)[:, :, 0:1].squeeze(2)

        y_tiles = []
        for ci in range(3):
            y = work.tile([64, JCH], fp32, name=f"y{ci}", tag=f"y{ci}")
            nc.vector.tensor_scalar(
                y[:, :], xn_tap(0),
                cw_sb[:, 10 * ci : 10 * ci + 1], cw_sb[:, 30 + ci : 31 + ci],
                OP.mult, OP.add,
            )
            for tp in range(1, 10):
                tmp = work.tile([64, JCH], fp32, name=f"tmp{ci}_{tp}", tag="convtmp")
                nc.vector.tensor_scalar(
                    tmp[:, :], xn_tap(tp),
                    cw_sb[:, 10 * ci + tp : 10 * ci + tp + 1], None, OP.mult,
                )
                nc.vector.tensor_tensor(y[:, :], y[:, :], tmp[:, :], OP.add)
            y_tiles.append(y)

        # ---- P1: transposes + QKV matmuls (lin_b rides lw row 998) ----
        yT = [const.tile([JCH, 64], bf16, name=f"yT{n}") for n in "qkv"]
        for ci in range(3):
            tr = ps_sm.tile([JCH, 64], fp32, name=f"tr{ci}", tag="small")
            nc.tensor.transpose(tr[:, :], y_tiles[ci][:, :], ident[0:64, 0:64])
            nc.vector.tensor_copy(yT[ci][:, :], tr[:, :])
            # bias lanes: j=998 -> 1 (carries lin_b via lw row 998), j=999 -> 0
            vfix = (
                yT[ci][123:125, :]
                .rearrange("p (b jc) -> p b jc", jc=8)[:, :, 7:8]
                .squeeze(2)
            )
            nc.sync.dma_start(out=vfix, in_=yfix_d[:, :])

        def yT_cols(ci, jc):
            return (
                yT[ci][:, :]
                .rearrange("p (b jc) -> p b jc", jc=8)[:, :, jc : jc + 1]
                .squeeze(2)
            )

        rows_sb = {0: qr}
        for ci, name in enumerate("qkv"):
            acc = ps_big.tile([BPC, 1024], fp32, name=f"acc{name}", tag="big")
            for jc in range(NJC):
                lwt = lwp.tile([JCH, D], bf16, name=f"lw{name}{jc}", tag="lw")
                nc.sync.dma_start(
                    out=lwt[:, :], in_=lw_d[ci][JCH * jc : JCH * (jc + 1), :]
                )
                for n0, n1 in ((0, 512), (512, 1000)):
                    nc.tensor.matmul(
                        acc[:, n0:n1], yT_cols(ci, jc), lwt[:, n0:n1],
                        start=(jc == 0), stop=(jc == NJC - 1),
                    )
            dst = qr if ci == 0 else (kp if ci == 1 else lnv)
            # relu into final row tiles; k and v get folded below (in place)
            nc.scalar.activation(dst[:, :], acc[0:BPC, 0:D], AF.Relu)
            rows_sb[ci] = dst

        # ---- P1.5: row stats + folds ----
        sc3 = const.tile([BPC, 2], fp32, name="sc3")
        nc.vector.tensor_reduce(sc3[:, 0:1], qr[:, :], AX.X, OP.max)
        nc.vector.tensor_reduce(sc3[:, 1:2], kp[:, :], AX.X, OP.add)
        scps = ps_sm.tile([2, BPC], fp32, name="scps", tag="small")
        nc.tensor.transpose(scps[:, :], sc3[:, :], ident[0:BPC, 0:BPC])
        nc.vector.tensor_copy(scrow[:, :], scps[:, :])
        nc.vector.tensor_scalar(
            w0hb[:, :], scrow[0:1, :], w0 / (NN - 1.0), None, OP.mult
        )
        nc.vector.tensor_scalar(
            neghb[:, :], scrow[0:1, :], -1.0 / (NN - 1.0), None, OP.mult
        )
        nc.vector.reciprocal(invh[:, :], scrow[0:1, :])
        nc.vector.tensor_scalar(invh[:, :], invh[:, :], NN - 1.0, None, OP.mult)
        kshift = const.tile([BPC, 1], fp32, name="kshift")
        nc.vector.tensor_scalar(
            kshift[:, :], sc3[:, 1:2], -w1 / float(D), None, OP.mult
        )
        # relu-knot bias columns: bcs8[m, b] = -m*h_b (rows 1..62), row62=0,
        # row63=1.0 (with scale 0 the relu yields the constant ones row)
        bcl8 = ps_sm.tile([NN, BPC], fp32, name="bcl8", tag="small")
        nc.tensor.matmul(bcl8[:, :], mshift, neghb[:, :], start=True, stop=False)
        nc.tensor.matmul(bcl8[:, :], e63, ones8, start=False, stop=True)
        bcs8 = const.tile([NN, BPC], fp32, name="bcs8")
        nc.vector.tensor_copy(bcs8[:, :], bcl8[:, :])
        # k' rows = w1*Kr - w1*mean(Kr) (in place on kp)
        nc.vector.tensor_scalar(kp[:, :], kp[:, :], w1, kshift[:, :], OP.mult, OP.add)
        # lnv rows = Ln(alpha*Vr + beta) (in place on lnv)
        beta_sb = const.tile([BPC, 1], fp32, name="beta_sb")
        nc.vector.memset(beta_sb[:, :], beta)
        nc.scalar.activation(
            lnv[:, :], lnv[:, :], AF.Ln, bias=beta_sb[:, :], scale=alpha
        )

        # ---- P2: per-batch node eval + PL interpolation ----
        ctxps = ps_ctx.tile([JCH, 64], fp32, name="ctxps", tag="ctx")
        for b in range(BPC):
            qrow = rowp.tile([1, D], f32r, name=f"qrow{b}", tag="qrow")
            krow = rowp.tile([1, D], f32r, name=f"krow{b}", tag="krow")
            vrow = rowp.tile([1, D], f32r, name=f"vrow{b}", tag="vrow")
            nc.sync.dma_start(out=qrow[:, :], in_=qr[b : b + 1, :].bitcast(f32r))
            nc.sync.dma_start(out=krow[:, :], in_=kp[b : b + 1, :].bitcast(f32r))
            nc.sync.dma_start(out=vrow[:, :], in_=lnv[b : b + 1, :].bitcast(f32r))
            tdup = work.tile([1, 128], f32r, name=f"tdup{b}", tag="tdup")
            nc.vector.tensor_scalar(
                tdup[:, 0:64], mfull, w0hb[:, b : b + 1], b0, OP.mult, OP.add
            )
            nc.vector.tensor_scalar(
                tdup[:, 64:128], mfull, w0hb[:, b : b + 1], b0, OP.mult, OP.add
            )
            nps = ps_data.tile([128, D], fp32, name=f"nps{b}", tag="data")
            for n0, n1 in ((0, 512), (512, 1000)):
                nc.tensor.matmul(
                    nps[:, n0:n1], tdup[:, :], krow[:, n0:n1],
                    start=True, stop=False,
                )
                nc.tensor.matmul(
                    nps[:, n0:n1], nmask, vrow[:, n0:n1],
                    start=False, stop=True,
                )
            nex = nexp.tile([128, D], bf16, name=f"nex{b}", tag="nex")
            ndc = work.tile([128, 1], fp32, name=f"ndc{b}", tag="ndc")
            nc.scalar.activation(nex[:, :], nps[:, :], AF.Exp, accum_out=ndc[:, :])
            ndr = ps_sm.tile([1, 128], fp32, name=f"ndr{b}", tag="small")
            nc.tensor.transpose(ndr[:, :], ndc[:, :], ident[:, :])

            # relu-basis stack: rows 0..61 = relu(r - m*h), 62 = r, 63 = ones
            rps = ps_data.tile([NN, D], fp32, name=f"rps{b}", tag="data")
            for n0, n1 in ((0, 512), (512, 1000)):
                nc.tensor.matmul(
                    rps[:, n0:n1], ones64, qrow[:, n0:n1],
                    start=True, stop=True,
                )
            rst = nexp.tile([NN, D], fp32, name=f"rst{b}", tag="rst")
            nc.scalar.activation(
                rst[:, :], rps[:, :], AF.Relu,
                bias=bcs8[:, b : b + 1], scale=scl64,
            )

            # f' nodes and PL coefficients
            frec = work.tile([1, NN], fp32, name=f"frec{b}", tag="frec")
            nc.vector.reciprocal(frec[:, :], ndr[:, 64:128])
            fpr = work.tile([1, NN], fp32, name=f"fpr{b}", tag="fpr")
            nc.vector.tensor_tensor(fpr[:, :], ndr[:, 0:64], frec[:, :], OP.mult)
            nc.vector.tensor_scalar(fpr[:, :], fpr[:, :], fs1, fs2, OP.mult, OP.add)
            d1 = work.tile([1, NN - 1], fp32, name=f"d1{b}", tag="d1")
            nc.vector.tensor_tensor(
                d1[:, :], fpr[:, 1:NN], fpr[:, 0 : NN - 1], OP.subtract
            )
            grow = work.tile([1, NN], fp32, name=f"grow{b}", tag="grow")
            nc.vector.tensor_tensor(
                grow[:, 0:NK], d1[:, 1 : NN - 1], d1[:, 0:NK], OP.subtract
            )
            nc.vector.tensor_scalar(
                grow[:, 0:NK], grow[:, 0:NK], invh[:, b : b + 1], None, OP.mult
            )
            nc.vector.tensor_scalar(
                grow[:, NK : NK + 1], d1[:, 0:1], invh[:, b : b + 1], None, OP.mult
            )
            nc.vector.tensor_copy(grow[:, NN - 1 : NN], fpr[:, 0:1])
            gps = ps_sm.tile([NN, 1], fp32, name=f"gps{b}", tag="small")
            nc.tensor.transpose(gps[:, :], grow[:, :], ident[0:1, 0:1])
            gcl = work.tile([NN, 1], fp32, name=f"gcl{b}", tag="gcl")
            nc.vector.tensor_copy(gcl[:, :], gps[:, :])
            for ic in range(NJC):
                col = 8 * ic + b
                nc.tensor.matmul(
                    ctxps[:, col : col + 1],
                    rst[:, JCH * ic : JCH * (ic + 1)], gcl[:, :],
                    start=True, stop=True,
                )

        # ---- P3: output matmul + bias ----
        nc.vector.tensor_copy(ctxT[:, :], ctxps[:, :])
        o_ps = ps_big.tile([BPC, 1024], fp32, name="o_ps", tag="big")
        for ic in range(NJC):
            owt = lwp.tile([JCH, D], bf16, name=f"ow{ic}", tag="lw")
            nc.sync.dma_start(
                out=owt[:, :], in_=owt_d[JCH * ic : JCH * (ic + 1), :]
            )
            for n0, n1 in ((0, 512), (512, 1000)):
                nc.tensor.matmul(
                    o_ps[:, n0:n1], ctxT[:, 8 * ic : 8 * ic + 8], owt[:, n0:n1],
                    start=(ic == 0), stop=(ic == NJC - 1),
                )
        out_sb = const.tile([BPC, D], fp32, name="out_sb")
        nc.vector.tensor_tensor(out_sb[:, :], o_ps[0:BPC, 0:D], outb_sb[:, :], OP.add)
        nc.sync.dma_start(out=out_d[:, :], in_=out_sb[:, :])

    nc.compile()
    return nc


def _fold_scalars(inputs):
    w = inputs["in_proj_w"].reshape(3).astype(np.float64)
    bb = inputs["in_proj_b"].reshape(3).astype(np.float64)
    ow = float(inputs["out_proj_w"].reshape(()))
    obp = float(inputs["out_proj_b"].reshape(()))
    w2, b2 = float(w[2]), float(bb[2])
    # v = w2*Vr + b2; need ln of a guaranteed-positive m = vsign*v + C
    vsign = 1.0 if (w2 > 0 or (w2 == 0 and b2 >= 0)) else -1.0
    if vsign * b2 > 0:
        C = 0.0
        beta = vsign * b2
    else:
        eps = 1e-3 * max(abs(b2), 1e-2)
        C = -vsign * b2 + eps
        beta = eps
    alpha = abs(w2)
    # ctx' = ow*(vsign*(n/d) - vsign*C) + obp  [n computed with +C shift]
    fs1 = ow * vsign
    fs2 = obp - ow * vsign * C
    return {
        "w0": float(w[0]), "b0": float(bb[0]), "w1": float(w[1]),
        "alpha": alpha, "beta": beta, "fs1": fs1, "fs2": fs2,
    }


def _prep_inputs(inputs):
    import ml_dtypes
    f32 = np.float32
    bf = ml_dtypes.bfloat16
    x = np.ascontiguousarray(inputs["x"].reshape(B, L_IN)).astype(f32, copy=False)
    s = (inputs["bn_gamma"] / np.sqrt(inputs["bn_var"] + EPS)).astype(f32)
    t = (inputs["bn_beta"] - inputs["bn_mean"] * s).astype(f32)

    idx = (500 * np.arange(NJC))[:, None] + np.arange(W)[None, :]
    x_pad = np.zeros((B, PADL), f32)
    x_pad[:, :L_IN] = x
    xw = x_pad[:, idx]

    s_pad = np.zeros(PADL, f32)
    s_pad[:L_IN] = s
    t_pad = np.zeros(PADL, f32)
    t_pad[:L_IN] = t
    s_t = np.tile(s_pad[idx], (BPC, 1))
    t_t = np.tile(t_pad[idx], (BPC, 1))

    cw = np.zeros(48, f32)
    for ci, n in enumerate("qkv"):
        cw[10 * ci : 10 * ci + 10] = inputs[f"conv_w_{n}"].reshape(10)
        cw[30 + ci] = inputs[f"conv_b_{n}"].reshape(())
    cw[40] = 1.0
    cw = np.tile(cw, (64, 1))
    cw[63, 40] = 0.0  # relu-scale col: row 63 becomes the constant ones row

    lw = {}
    for ci, n in enumerate("qkv"):
        m = np.zeros((1000, D), f32)
        m[:LC, :] = inputs[f"lin_w_{n}"].T
        m[998, :] = inputs[f"lin_b_{n}"]
        lw[n] = np.ascontiguousarray(m.astype(bf))

    owt = np.ascontiguousarray(inputs["out_w"].T.astype(bf))
    ob8 = np.tile(inputs["out_b"].astype(f32), (BPC, 1))

    crows = np.zeros((1, 320), f32)
    crows[0, 0:64] = np.arange(64)
    crows[0, 64 : 64 + 62] = np.arange(1, 63)
    crows[0, 128 : 128 + 64] = 1.0
    crows[0, 319] = 1.0
    import ml_dtypes as _md
    yfix = np.zeros((2, 8), _md.bfloat16)
    yfix[0, :] = 1.0

    shared = {
        "cw": np.ascontiguousarray(cw),
        "lwq": lw["q"], "lwk": lw["k"], "lwv": lw["v"],
        "owt": owt,
        "ob": np.ascontiguousarray(ob8),
        "crows": crows,
        "yfix": yfix,
    }
    in_maps = []
    for c in range(NCORE):
        m = dict(shared)
        xc = xw[BPC * c : BPC * (c + 1)].reshape(64, W)
        m["xst"] = np.ascontiguousarray(np.concatenate([xc, s_t, t_t], axis=1))
        in_maps.append(m)
    return in_maps


def kernel(**inputs) -> np.ndarray:
    from concourse.bass_utils import run_bass_kernel_spmd

    sc = _fold_scalars(inputs)
    key = tuple(sorted(sc.items()))
    if _CACHE.get("key") != key:
        _CACHE["nc"] = _build_module(sc)
        _CACHE["key"] = key
    nc = _CACHE["nc"]
    in_maps = _prep_inputs(inputs)
    res = run_bass_kernel_spmd(nc, in_maps, list(range(NCORE)))
    outs = [res.results[c]["out"] for c in range(NCORE)]
    return np.concatenate(outs, axis=0).reshape(B, 1, D).astype(np.float32)


# revision 16
# speedup vs baseline: 1.5092x; 1.1114x over previous
"""Trainium2 Bass kernel for nn_ConvAttn (BN -> 3x(conv1d+linear+relu) -> scalar MHA -> linear).

Sharding: pure data parallel, batch 64 -> 8 cores x 8 batches.

Key insight: with embed_dim=1 the attention context for query i depends on q_i
only through the scalar q_i:
    ctx_i = f(q_i),  f(t) = sum_j v_j e^{t k'_j} / sum_j e^{t k'_j}
(k' = k - mean(k); the shift cancels in softmax). f is smooth, so instead of
the 1000x1000 score matrix per batch we evaluate f at 16 nodes t_m spanning
the realized q-range (per batch) and piecewise-linearly interpolate. PL in the
relu basis: f_PL(r) = f_0 + s_0 r + sum_m g_m relu(r - rho_m), so per-i
evaluation is ONE tiny matmul against a relu-basis matrix built by one ACT
Relu. Offline validation: final rel err ~1e-3 vs exact (T=14 interior knots).

All 8 batches are stacked on the partition dim (16 nodes x 8 batches = 128),
so the node evaluation for the WHOLE CORE is: 3 one-hot K=8 matmuls to build
outer products t_m*k'_j (+ln|v_j|) -> 2 ACT Exp with accum_out -> node sums.

Device pipeline per core (8 batches):
  P0: BN + 10-tap conv on DVE -> y [64,125] x3 (+ bias-lane trick: y-lane
      998=1, 999=0 via tiny DMA; lw row 998 = lin_b)
  P1: PE transpose y -> yT; QKV linear (weights moving, bf16) -> [8,1000]
      psum; ACT Relu -> Qr/Kr/Vr rows [8,1000]
  P2: row stats -> grid scalars; k'-fold; ln|v| row (ACT Ln);
      toh[b,p]=delta*t_{m(p),b}; num/den [128,1000] psum (fp32 K=8 matmuls);
      2x ACT Exp + accum -> node sums [128,1]; transpose -> f-row [1,128];
      per batch ~6 tiny DVE ops -> g column [128,1] (zero outside own block);
      relu-basis [128,1000] (1 matmul + 1 ACT Relu, scale/bias per partition);
      64 4-cycle matmuls -> ctx columns
  P3: ctx -> bf16 ctxT [125,64]; out matmul (owt moving bf16) + bias; DMA out
"""

import numpy as np

B = 64
L_IN = 4001
LC = 998
D = 1000
EPS = 1e-5
NCORE = 8
BPC = B // NCORE            # 8 batches per core
PADL = 4016                 # padded x row length
W = 512                     # conv input window per jc chunk
NJC = 8                     # position chunks, stride 500
JCH = 125                   # conv outputs per chunk; 8*125 = 1000 lanes
NN = 16                     # interpolation nodes per batch (grid [0, rmax])

_CACHE = {}


def _build_module(sc):
    import concourse.bacc as bacc
    import concourse.mybir as mybir
    import concourse.tile as tile
    from concourse.masks import make_identity
    from contextlib import ExitStack

    fp32 = mybir.dt.float32
    bf16 = mybir.dt.bfloat16
    AF = mybir.ActivationFunctionType
    OP = mybir.AluOpType
    AX = mybir.AxisListType

    w0, b0 = sc["w0"], sc["b0"]
    w1 = sc["w1"]
    alpha, beta = sc["alpha"], sc["beta"]     # ln arg = alpha*Vr + beta
    fs1, fs2 = sc["fs1"], sc["fs2"]           # f' = fs1*(n/d) + fs2

    nc = bacc.Bacc()
    xst_d = nc.declare_dram_parameter("xst", [64, 3 * W], fp32, isOutput=False)
    cw_d = nc.declare_dram_parameter("cw", [64, 40], fp32, isOutput=False)
    lw_d = [
        nc.declare_dram_parameter(f"lw{n}", [1000, D], bf16, isOutput=False)
        for n in "qkv"
    ]
    owt_d = nc.declare_dram_parameter("owt", [D, D], bf16, isOutput=False)
    ob_d = nc.declare_dram_parameter("ob", [BPC, D], fp32, isOutput=False)
    # oneh [8 x 384]: [0:128]=onehot  [128:256]=m(p)*onehot  [256:384]=b0*onehot
    oh_d = nc.declare_dram_parameter("oneh", [BPC, 384], fp32, isOutput=False)
    # pcol [128, 4]: col0=relu scale (0 at idx15), col1=knot idx (1..14,0,0), col2=ones@idx15
    pc_d = nc.declare_dram_parameter("pcol", [128, 4], fp32, isOutput=False)
    yfix_d = nc.declare_dram_parameter("yfix", [2, 8], bf16, isOutput=False)
    out_d = nc.declare_dram_parameter("out", [BPC, D], fp32, isOutput=True)

    with tile.TileContext(nc) as tc, ExitStack() as ctx:
        const = ctx.enter_context(tc.tile_pool(name="const", bufs=1))
        work = ctx.enter_context(tc.tile_pool(name="work", bufs=3))
        lwp = ctx.enter_context(tc.tile_pool(name="lwp", bufs=2))
        nexp = ctx.enter_context(tc.tile_pool(name="nexp", bufs=2))
        gw = ctx.enter_context(tc.tile_pool(name="gw", bufs=3))
        # PSUM (8 banks): data 2x2 + big(qkv/out) 2 + small 2x... + ctx 1
        ps_data = ctx.enter_context(tc.tile_pool(name="ps_data", bufs=2, space="PSUM"))
        ps_big = ctx.enter_context(tc.tile_pool(name="ps_big", bufs=1, space="PSUM"))
        ps_sm = ctx.enter_context(tc.tile_pool(name="ps_sm", bufs=1, space="PSUM"))
        ps_ctx = ctx.enter_context(tc.tile_pool(name="ps_ctx", bufs=1, space="PSUM"))

        # ---- constants ----
        xst = const.tile([64, 3 * W], fp32)
        cw_sb = const.tile([64, 40], fp32)
        oneh = const.tile([BPC, 384], fp32)
        pcol = const.tile([128, 4], fp32)
        ident = const.tile([128, 128], fp32)
        nc.sync.dma_start(out=xst[:, :], in_=xst_d[:, :])
        nc.sync.dma_start(out=cw_sb[:, :], in_=cw_d[:, :])
        nc.sync.dma_start(out=oneh[:, :], in_=oh_d[:, :])
        nc.sync.dma_start(out=pcol[:, :], in_=pc_d[:, :])
        make_identity(nc, ident[:, :])
        ohot = oneh[:, 0:128]
        ohA = oneh[:, 128:256]
        ohB = oneh[:, 256:384]
        scl_col = pcol[:, 0:1]
        msh_col = pcol[:, 1:2]
        eon_col = pcol[:, 2:3]
        outb_sb = const.tile([BPC, D], fp32)
        nc.sync.dma_start(out=outb_sb[:, :], in_=ob_d[:, :])

        ctxT = const.tile([JCH, 64], bf16)
        qr = const.tile([BPC, D], fp32, name="qr")
        kp = const.tile([BPC, D], fp32, name="kp")
        lnv = const.tile([BPC, D], fp32, name="lnv")

        # ---- P0: BN + conv ----
        xn = const.tile([64, W], fp32)
        nc.vector.tensor_tensor(xn[:, :], xst[:, 0:W], xst[:, W : 2 * W], OP.mult)
        nc.vector.tensor_tensor(xn[:, :], xn[:, :], xst[:, 2 * W : 3 * W], OP.add)

        def xn_tap(tp):
            v = xn[:, tp : tp + 500]
            return v.rearrange("p (j f) -> p j f", f=4)[:, :, 0:1].squeeze(2)

        y_tiles = []
        for ci in range(3):
            y = work.tile([64, JCH], fp32, name=f"y{ci}", tag=f"y{ci}")
            nc.vector.tensor_scalar(
                y[:, :], xn_tap(0),
                cw_sb[:, 10 * ci : 10 * ci + 1], cw_sb[:, 30 + ci : 31 + ci],
                OP.mult, OP.add,
            )
            for tp in range(1, 10):
                tmp = work.tile([64, JCH], fp32, name=f"tmp{ci}_{tp}", tag="convtmp")
                nc.vector.tensor_scalar(
                    tmp[:, :], xn_tap(tp),
                    cw_sb[:, 10 * ci + tp : 10 * ci + tp + 1], None, OP.mult,
                )
                nc.vector.tensor_tensor(y[:, :], y[:, :], tmp[:, :], OP.add)
            y_tiles.append(y)

        # ---- P1: transposes + QKV matmuls (lin_b rides lw row 998) ----
        yT = [const.tile([JCH, 64], bf16, name=f"yT{n}") for n in "qkv"]
        for ci in range(3):
            tr = ps_sm.tile([JCH, 64], fp32, name=f"tr{ci}", tag="small")
            nc.tensor.transpose(tr[:, :], y_tiles[ci][:, :], ident[0:64, 0:64])
            nc.vector.tensor_copy(yT[ci][:, :], tr[:, :])
            vfix = (
                yT[ci][123:125, :]
                .rearrange("p (b jc) -> p b jc", jc=8)[:, :, 7:8]
                .squeeze(2)
            )
            nc.sync.dma_start(out=vfix, in_=yfix_d[:, :])

        def yT_cols(ci, jc):
            return (
                yT[ci][:, :]
                .rearrange("p (b jc) -> p b jc", jc=8)[:, :, jc : jc + 1]
                .squeeze(2)
            )

        for ci, name in enumerate("qkv"):
            acc = ps_big.tile([BPC, 1024], fp32, name=f"acc{name}", tag="big")
            for jc in range(NJC):
                lwt = lwp.tile([JCH, D], bf16, name=f"lw{name}{jc}", tag="lw")
                nc.sync.dma_start(
                    out=lwt[:, :], in_=lw_d[ci][JCH * jc : JCH * (jc + 1), :]
                )
                for n0, n1 in ((0, 512), (512, 1000)):
                    nc.tensor.matmul(
                        acc[:, n0:n1], yT_cols(ci, jc), lwt[:, n0:n1],
                        start=(jc == 0), stop=(jc == NJC - 1),
                    )
            dst = qr if ci == 0 else (kp if ci == 1 else lnv)
            nc.scalar.activation(dst[:, :], acc[0:BPC, 0:D], AF.Relu)

        # ---- P2a: row stats + folds + grid scalars ----
        sc3 = const.tile([BPC, 2], fp32, name="sc3")
        nc.vector.tensor_reduce(sc3[:, 0:1], qr[:, :], AX.X, OP.max)
        nc.vector.tensor_reduce(sc3[:, 1:2], kp[:, :], AX.X, OP.add)
        w0hb = const.tile([BPC, 1], fp32, name="w0hb")
        neghb = const.tile([BPC, 1], fp32, name="neghb")
        nc.vector.tensor_scalar(w0hb[:, :], sc3[:, 0:1], w0 / (NN - 1.0), None, OP.mult)
        nc.vector.tensor_scalar(
            neghb[:, :], sc3[:, 0:1], -1.0 / (NN - 1.0), None, OP.mult
        )
        rmr = ps_sm.tile([1, BPC], fp32, name="rmr", tag="small")
        nc.tensor.transpose(rmr[:, :], sc3[:, 0:1], ident[0:BPC, 0:BPC])
        invh = const.tile([1, BPC], fp32, name="invh")
        nc.vector.reciprocal(invh[:, :], rmr[:, :])
        nc.vector.tensor_scalar(invh[:, :], invh[:, :], NN - 1.0, None, OP.mult)

        kshift = const.tile([BPC, 1], fp32, name="kshift")
        nc.vector.tensor_scalar(
            kshift[:, :], sc3[:, 1:2], -w1 / float(D), None, OP.mult
        )
        nc.vector.tensor_scalar(kp[:, :], kp[:, :], w1, kshift[:, :], OP.mult, OP.add)
        beta_sb = const.tile([BPC, 1], fp32, name="beta_sb")
        nc.vector.memset(beta_sb[:, :], beta)
        nc.scalar.activation(
            lnv[:, :], lnv[:, :], AF.Ln, bias=beta_sb[:, :], scale=alpha
        )

        # toh[b, p] = onehot * t_{m(p), b};  t = m*w0*h_b + b0
        toh = const.tile([BPC, 128], fp32, name="toh")
        nc.vector.tensor_scalar(toh[:, :], ohA, w0hb[:, :], None, OP.mult)
        nc.vector.tensor_tensor(toh[:, :], toh[:, :], ohB, OP.add)

        # relu-knot bias column: bcs[p] = -m'(p)*h_b(p) + ones@idx15
        hexp = ps_sm.tile([128, 1], fp32, name="hexp", tag="small")
        nc.tensor.matmul(hexp[:, :], ohot, neghb[:, :], start=True, stop=True)
        bcs = const.tile([128, 1], fp32, name="bcs")
        nc.vector.tensor_tensor(bcs[:, :], msh_col, hexp[:, :], OP.mult)
        nc.vector.tensor_tensor(bcs[:, :], bcs[:, :], eon_col, OP.add)

        # ---- P2b: node inputs, exps, relu basis ----
        nmm = ps_data.tile([128, D], fp32, name="nmm", tag="data")
        dmm = ps_data.tile([128, D], fp32, name="dmm", tag="data")
        for n0, n1 in ((0, 512), (512, 1000)):
            nc.tensor.matmul(
                nmm[:, n0:n1], toh[:, :], kp[:, n0:n1], start=True, stop=False
            )
            nc.tensor.matmul(
                nmm[:, n0:n1], ohot, lnv[:, n0:n1], start=False, stop=True
            )
            nc.tensor.matmul(
                dmm[:, n0:n1], toh[:, :], kp[:, n0:n1], start=True, stop=True
            )
        nexN = nexp.tile([128, D], bf16, name="nexN", tag="nex")
        nexD = nexp.tile([128, D], bf16, name="nexD", tag="nex")
        accN = const.tile([128, 1], fp32, name="accN")
        accD = const.tile([128, 1], fp32, name="accD")
        nc.scalar.activation(nexN[:, :], nmm[:, :], AF.Exp, accum_out=accN[:, :])
        nc.scalar.activation(nexD[:, :], dmm[:, :], AF.Exp, accum_out=accD[:, :])

        rps = ps_data.tile([128, D], fp32, name="rps", tag="data")
        for n0, n1 in ((0, 512), (512, 1000)):
            nc.tensor.matmul(
                rps[:, n0:n1], ohot, qr[:, n0:n1], start=True, stop=True
            )
        rstack = const.tile([128, D], fp32, name="rstack")
        nc.scalar.activation(
            rstack[:, :], rps[:, :], AF.Relu, bias=bcs[:, :], scale=scl_col
        )

        # ---- P2c: node values f -> PL coefficients per batch ----
        trN = ps_sm.tile([1, 128], fp32, name="trN", tag="small")
        nc.tensor.transpose(trN[:, :], accN[:, :], ident[:, :])
        nrow = const.tile([1, 128], fp32, name="nrow")
        nc.vector.tensor_copy(nrow[:, :], trN[:, :])
        trD = ps_sm.tile([1, 128], fp32, name="trD", tag="small")
        nc.tensor.transpose(trD[:, :], accD[:, :], ident[:, :])
        recD = const.tile([1, 128], fp32, name="recD")
        frow = const.tile([1, 128], fp32, name="frow")
        nc.vector.reciprocal(recD[:, :], trD[:, :])
        nc.vector.tensor_tensor(frow[:, :], nrow[:, :], recD[:, :], OP.mult)
        nc.vector.tensor_scalar(frow[:, :], frow[:, :], fs1, fs2, OP.mult, OP.add)

        ctxps = ps_ctx.tile([JCH, 64], fp32, name="ctxps", tag="ctx")
        for b in range(BPC):
            s = NN * b
            fb = frow[:, s : s + NN]
            d1 = gw.tile([1, NN - 1], fp32, name=f"d1{b}", tag="d1")
            nc.vector.tensor_tensor(
                d1[:, :], fb[:, 1:NN], fb[:, 0 : NN - 1], OP.subtract
            )
            grow = gw.tile([1, 128], fp32, name=f"grow{b}", tag="grow")
            nc.vector.memset(grow[:, :], 0.0)
            nc.vector.tensor_tensor(
                grow[:, s : s + NN - 2], d1[:, 1 : NN - 1], d1[:, 0 : NN - 2],
                OP.subtract,
            )
            nc.vector.tensor_scalar(
                grow[:, s : s + NN - 2], grow[:, s : s + NN - 2],
                invh[:, b : b + 1], None, OP.mult,
            )
            nc.vector.tensor_scalar(
                grow[:, s + NN - 2 : s + NN - 1], d1[:, 0:1],
                invh[:, b : b + 1], None, OP.mult,
            )
            nc.vector.tensor_copy(grow[:, s + NN - 1 : s + NN], fb[:, 0:1])
            gps = ps_sm.tile([128, 1], fp32, name=f"gps{b}", tag="small")
            nc.tensor.transpose(gps[:, :], grow[:, :], ident[0:1, 0:1])
            gcl = gw.tile([128, 1], fp32, name=f"gcl{b}", tag="gcl")
            nc.vector.tensor_copy(gcl[:, :], gps[:, :])
            for ic in range(NJC):
                col = 8 * ic + b
                nc.tensor.matmul(
                    ctxps[:, col : col + 1],
                    rstack[:, JCH * ic : JCH * (ic + 1)], gcl[:, :],
                    start=True, stop=True,
                )

        # ---- P3: output matmul + bias ----
        nc.vector.tensor_copy(ctxT[:, :], ctxps[:, :])
        o_ps = ps_big.tile([BPC, 1024], fp32, name="o_ps", tag="big")
        for ic in range(NJC):
            owt = lwp.tile([JCH, D], bf16, name=f"ow{ic}", tag="lw")
            nc.sync.dma_start(
                out=owt[:, :], in_=owt_d[JCH * ic : JCH * (ic + 1), :]
            )
            for n0, n1 in ((0, 512), (512, 1000)):
                nc.tensor.matmul(
                    o_ps[:, n0:n1], ctxT[:, 8 * ic : 8 * ic + 8], owt[:, n0:n1],
                    start=(ic == 0), stop=(ic == NJC - 1),
                )
        out_sb = const.tile([BPC, D], fp32, name="out_sb")
        nc.vector.tensor_tensor(out_sb[:, :], o_ps[0:BPC, 0:D], outb_sb[:, :], OP.add)
        nc.sync.dma_start(out=out_d[:, :], in_=out_sb[:, :])

    nc.compile()
    return nc


def _fold_scalars(inputs):
    w = inputs["in_proj_w"].reshape(3).astype(np.float64)
    bb = inputs["in_proj_b"].reshape(3).astype(np.float64)
    ow = float(inputs["out_proj_w"].reshape(()))
    obp = float(inputs["out_proj_b"].reshape(()))
    w2, b2 = float(w[2]), float(bb[2])
    # v = w2*Vr + b2; need ln of a guaranteed-positive m = vsign*v + C
    vsign = 1.0 if (w2 > 0 or (w2 == 0 and b2 >= 0)) else -1.0
    if vsign * b2 > 0:
        C = 0.0
        beta = vsign * b2
    else:
        eps = 1e-3 * max(abs(b2), 1e-2)
        C = -vsign * b2 + eps
        beta = eps
    alpha = abs(w2)
    fs1 = ow * vsign
    fs2 = obp - ow * vsign * C
    return {
        "w0": float(w[0]), "b0": float(bb[0]), "w1": float(w[1]),
        "alpha": alpha, "beta": beta, "fs1": fs1, "fs2": fs2,
    }


def _prep_inputs(inputs):
    import ml_dtypes
    f32 = np.float32
    bf = ml_dtypes.bfloat16
    sc = _fold_scalars(inputs)
    x = np.ascontiguousarray(inputs["x"].reshape(B, L_IN)).astype(f32, copy=False)
    s = (inputs["bn_gamma"] / np.sqrt(inputs["bn_var"] + EPS)).astype(f32)
    t = (inputs["bn_beta"] - inputs["bn_mean"] * s).astype(f32)

    idx = (500 * np.arange(NJC))[:, None] + np.arange(W)[None, :]
    x_pad = np.zeros((B, PADL), f32)
    x_pad[:, :L_IN] = x
    xw = x_pad[:, idx]

    s_pad = np.zeros(PADL, f32)
    s_pad[:L_IN] = s
    t_pad = np.zeros(PADL, f32)
    t_pad[:L_IN] = t
    s_t = np.tile(s_pad[idx], (BPC, 1))
    t_t = np.tile(t_pad[idx], (BPC, 1))

    cw = np.zeros(40, f32)
    for ci, n in enumerate("qkv"):
        cw[10 * ci : 10 * ci + 10] = inputs[f"conv_w_{n}"].reshape(10)
        cw[30 + ci] = inputs[f"conv_b_{n}"].reshape(())
    cw = np.tile(cw, (64, 1))

    lw = {}
    for ci, n in enumerate("qkv"):
        m = np.zeros((1000, D), f32)
        m[:LC, :] = inputs[f"lin_w_{n}"].T
        m[998, :] = inputs[f"lin_b_{n}"]
        lw[n] = np.ascontiguousarray(m.astype(bf))

    owt = np.ascontiguousarray(inputs["out_w"].T.astype(bf))
    ob8 = np.tile(inputs["out_b"].astype(f32), (BPC, 1))

    p = np.arange(128)
    onehot = (p[None, :] // NN == np.arange(BPC)[:, None]).astype(f32)
    oneh = np.concatenate(
        [onehot, (p[None, :] % NN) * onehot, sc["b0"] * onehot], axis=1
    ).astype(f32)

    pcol = np.zeros((128, 4), f32)
    idxp = p % NN
    pcol[:, 0] = (idxp != NN - 1).astype(f32)             # relu scale
    pcol[:, 1] = np.where(idxp <= NN - 3, idxp + 1, 0.0)  # knot index 1..14
    pcol[:, 2] = (idxp == NN - 1).astype(f32)             # ones row bias

    yfix = np.zeros((2, 8), bf)
    yfix[0, :] = 1.0

    shared = {
        "cw": np.ascontiguousarray(cw),
        "lwq": lw["q"], "lwk": lw["k"], "lwv": lw["v"],
        "owt": owt,
        "ob": np.ascontiguousarray(ob8),
        "oneh": np.ascontiguousarray(oneh),
        "pcol": pcol,
        "yfix": yfix,
    }
    in_maps = []
    for c in range(NCORE):
        m = dict(shared)
        xc = xw[BPC * c : BPC * (c + 1)].reshape(64, W)
        m["xst"] = np.ascontiguousarray(np.concatenate([xc, s_t, t_t], axis=1))
        in_maps.append(m)
    return in_maps


def kernel(**inputs) -> np.ndarray:
    from concourse.bass_utils import run_bass_kernel_spmd

    sc = _fold_scalars(inputs)
    key = tuple(sorted(sc.items()))
    if _CACHE.get("key") != key:
        _CACHE["nc"] = _build_module(sc)
        _CACHE["key"] = key
    nc = _CACHE["nc"]
    in_maps = _prep_inputs(inputs)
    res = run_bass_kernel_spmd(nc, in_maps, list(range(NCORE)))
    outs = [res.results[c]["out"] for c in range(NCORE)]
    return np.concatenate(outs, axis=0).reshape(B, 1, D).astype(np.float32)
